# revision 16
# baseline (speedup 1.0000x reference)
"""Trainium2 Bass kernel for Conv2d: B=16, Cin=Cout=16, H=W=512, k=3, stride=1, pad=1.

Strategy:
  - Data-parallel over batch: 8 cores x 2 images each. Weights/bias replicated.
  - Per core the conv is a sequence of TensorEngine matmuls in an H-Toeplitz
    packing: contraction K = 16 ci x 8 input rows = 128, stationary
    M = 16 co x 6 output rows = 96, moving N = 512 w-pixels. Each chunk of 6
    output rows takes 3 matmuls (one per kw tap, column-shifted rhs)
    accumulating into one PSUM bank; kh lives inside the Toeplitz stationary.
  - Mixed-precision matmul: stationary weights fp16, moving x fp8 e3m4
    (1 cycle/col on PE either way). e3m4 x keeps rel-err ~1.3e-2 < 2e-2
    while halving input HBM traffic; output written back as fp16 (halves
    output traffic), upcast to fp32 on host.
  - Host-side gathered DRAM layouts:
      xg[b, ci, hi, j, w'] = xpad[b, ci, 6j+hi, w']   (8/6 row duplication)
      yg[b, co, ho, j, w]  -> y[b, co, 6j+ho, w]      (scattered back on host)
    so chunk-major group DMAs read/write multi-chunk contiguous runs per
    partition (grp=16 chunks per DMA => 8-16 KB descriptors per SDMA engine).
  - Partition layouts are channel-major (ci*8+hi / co*6+ho) and every DMA's
    DRAM-side outer dim is the 16-entry channel dim -> the HWDGE spreads each
    transfer across all 16 SDMA engines.
  - Input DMAs ride the sync HWDGE ring, output DMAs the scalar HWDGE ring.
  - Matmuls issue kw-major inside a sub-round of 8 chunks (all chunks' kw=1,
    then kw=0, then kw=2) so the stationary switches 3x per 8 chunks; the 8
    open PSUM accumulation groups occupy all 8 banks (single-buffered).
  - PSUM->SBUF copy + bias add alternates between the vector engine
    (tensor_scalar_add) and the scalar engine (activation Identity+bias) so
    neither engine gates the PE.
"""

import numpy as np

B, CIN, COUT, H, W = 16, 16, 16, 512, 512
NCORES = 8
BPC = B // NCORES  # images per core
T_OUT, T_IN = 6, 8
KP, MP = T_IN * CIN, T_OUT * COUT  # 128, 96
NCHUNK = (H + T_OUT - 1) // T_OUT  # 86
WPAD = W + 2  # 514 padded cols

DEFAULT_CFG = dict(
    mm_dtype="fp16",      # stationary weights dtype
    x_dtype="fp8e3",      # moving x dtype (fp8 e3m4)
    out_dtype="fp16",     # y HBM dtype
    in_dma="sync",
    out_dma="scalar",
    grp=16,               # chunks per input DMA group
    sub=8,                # chunks per PSUM sub-round (8 banks)
    out_grp=8,            # chunks per output DMA
    bias_split=True,      # alternate PSUM->SBUF+bias between vector/scalar
)

_cached = {}


def _groups(grp):
    # NCHUNK = 86 = 6 + 5*16: put the remainder group FIRST so the first
    # input DMA is small and the PE starts sooner.
    rem = NCHUNK % grp
    out = []
    j = 0
    if rem:
        out.append((0, rem))
        j = rem
    while j < NCHUNK:
        out.append((j, grp))
        j += grp
    return out


def _dt(mybir, name):
    return {
        "fp32": mybir.dt.float32,
        "fp32r": mybir.dt.float32r,
        "fp16": mybir.dt.float16,
        "bf16": mybir.dt.bfloat16,
        "fp8e3": mybir.dt.float8e3,
        "fp8e4": mybir.dt.float8e4,
    }[name]


def _build_program(**overrides):
    cfg = dict(DEFAULT_CFG, **overrides)
    key = tuple(sorted(cfg.items()))
    if key in _cached:
        return _cached[key]
    import concourse.bacc as bacc
    import concourse.tile as tile
    import concourse.mybir as mybir

    nc = bacc.Bacc(
        "TRN2",
        target_bir_lowering=False,
        debug=False,
        enable_asserts=False,
        num_devices=NCORES,
    )
    f32 = mybir.dt.float32
    wdt = _dt(mybir, cfg["mm_dtype"])
    xdt = _dt(mybir, cfg["x_dtype"])
    ydt = _dt(mybir, cfg["out_dtype"])
    x = nc.dram_tensor(
        "x", [BPC, CIN, T_IN, NCHUNK, WPAD], xdt, kind="ExternalInput"
    ).ap()
    wt = nc.dram_tensor("wt", [KP, 3 * MP], wdt, kind="ExternalInput").ap()
    bias = nc.dram_tensor("bias", [MP, 1], f32, kind="ExternalInput").ap()
    y = nc.dram_tensor(
        "y", [BPC, COUT, T_OUT, NCHUNK, W], ydt, kind="ExternalOutput"
    ).ap()

    in_eng = getattr(nc, cfg["in_dma"])
    out_eng = getattr(nc, cfg["out_dma"])
    grp = cfg["grp"]
    sub = cfg["sub"]
    ogrp = cfg["out_grp"]

    with tile.TileContext(nc) as tc:
        with (
            tc.tile_pool(name="consts", bufs=1) as cpool,
            tc.tile_pool(name="xin", bufs=5) as xpool,
            tc.tile_pool(name="psum", bufs=1, space="PSUM") as ppool,
            tc.tile_pool(name="outs", bufs=10) as opool,
        ):
            # constants ride the scalar ring so the first X tile's DMA
            # dispatches immediately on the sync ring
            wt_sb = cpool.tile([KP, 3 * MP], wdt)
            out_eng.dma_start(wt_sb[:], wt[:])
            bias_sb = cpool.tile([MP, 1], f32)
            out_eng.dma_start(bias_sb[:], bias[:])

            for b in range(BPC):
                for j0, g in _groups(grp):
                    X = xpool.tile([KP, grp * WPAD], xdt, tag="X")
                    # partition (ci*8+hi) <- g chunks, contiguous per
                    # partition in the gathered DRAM layout
                    in_eng.dma_start(
                        X[:, 0 : g * WPAD],
                        x[b, :, :, j0 : j0 + g, :],
                    )
                    for s0 in range(0, g, sub):
                        sg = min(sub, g - s0)
                        pss = [
                            ppool.tile([MP, W], f32, tag=f"ps{k}", name=f"ps{k}")
                            for k in range(sg)
                        ]
                        for i, kw in enumerate((1, 0, 2)):
                            for k in range(sg):
                                gi = s0 + k
                                nc.tensor.matmul(
                                    pss[k][:, :],
                                    wt_sb[:, kw * MP : (kw + 1) * MP],
                                    X[:, gi * WPAD + kw : gi * WPAD + kw + W],
                                    start=(i == 0),
                                    stop=(i == 2),
                                )
                        out_sb = None
                        for k in range(sg):
                            if k % ogrp == 0:
                                out_sb = opool.tile([MP, ogrp * W], ydt, tag="out")
                                o0 = k
                            dst = out_sb[:, (k - o0) * W : (k - o0 + 1) * W]
                            if cfg["bias_split"] and k % 2 == 1:
                                nc.scalar.activation(
                                    dst,
                                    pss[k][:, :],
                                    mybir.ActivationFunctionType.Identity,
                                    bias=bias_sb[:, 0:1],
                                    scale=1.0,
                                )
                            else:
                                nc.vector.tensor_scalar_add(
                                    dst, pss[k][:, :], bias_sb[:, 0:1]
                                )
                            if k % ogrp == ogrp - 1 or k == sg - 1:
                                # partition (co*6+ho) -> yg[b, co, ho, ...]
                                og = k - o0 + 1
                                out_eng.dma_start(
                                    y[b, :, :, j0 + s0 + o0 : j0 + s0 + o0 + og, :],
                                    out_sb[:, 0 : og * W],
                                )
    nc.compile()
    _cached[key] = nc
    return nc


def _np_dt(name):
    import ml_dtypes

    return {
        "fp32": np.float32,
        "fp32r": np.float32,
        "fp16": np.float16,
        "bf16": ml_dtypes.bfloat16,
        "fp8e3": ml_dtypes.float8_e3m4,
        "fp8e4": ml_dtypes.float8_e4m3,
    }[name]


def _toeplitz_weights(weights: np.ndarray) -> np.ndarray:
    """[COUT, CIN, 3, 3] -> [KP, 3*MP] with K index ci*T_IN+hi and M index
    co*T_OUT+ho; lhsT_kw[ci*8+hi, co*6+ho] = W[co, ci, hi-ho, kw] for
    0 <= hi-ho <= 2, else 0. kw blocks side by side."""
    wt = np.zeros((3, CIN, T_IN, COUT, T_OUT), dtype=np.float32)
    for kw in range(3):
        for ho in range(T_OUT):
            for kh in range(3):
                wt[kw, :, ho + kh, :, ho] = weights[:, :, kh, kw].T
    wt2 = wt.reshape(3, KP, MP)
    return np.ascontiguousarray(np.concatenate([wt2[0], wt2[1], wt2[2]], axis=1))


def _make_in_maps(x, weights, biases, mm_dtype=None):
    cfg = DEFAULT_CFG
    wnp = _np_dt(cfg["mm_dtype"])
    xnp = _np_dt(cfg["x_dtype"])
    wt_packed = _toeplitz_weights(weights).astype(wnp)
    bias_vec = np.ascontiguousarray(np.repeat(biases, T_OUT).reshape(MP, 1))
    # zero-pad to [HP, WPAD] then gather rows: xg[b,ci,hi,j,w] = xp[b,ci,6j+hi,w]
    hp = T_OUT * NCHUNK + 2  # 518
    xp = np.zeros((B, CIN, hp, WPAD), dtype=xnp)
    xp[:, :, 1 : 1 + H, 1 : 1 + W] = x.astype(xnp)
    rows = np.arange(T_IN)[:, None] + T_OUT * np.arange(NCHUNK)[None, :]  # [8, 86]
    xg = xp[:, :, rows, :]  # [B, CIN, 8, 86, WPAD]
    return [
        {
            "x": np.ascontiguousarray(xg[k * BPC : (k + 1) * BPC]),
            "wt": wt_packed,
            "bias": bias_vec,
        }
        for k in range(NCORES)
    ]


def _gather_output(res_list):
    yg = np.concatenate(res_list, axis=0)  # [B, COUT, 6, NCHUNK, W]
    yfull = yg.transpose(0, 1, 3, 2, 4).reshape(B, COUT, NCHUNK * T_OUT, W)
    return yfull[:, :, :H, :].astype(np.float32)


def kernel(x, weights, biases):
    from concourse import bass_utils

    x = np.ascontiguousarray(np.asarray(x, dtype=np.float32))
    weights = np.asarray(weights, dtype=np.float32)
    biases = np.asarray(biases, dtype=np.float32)

    nc = _build_program()
    in_maps = _make_in_maps(x, weights, biases)
    res = bass_utils.run_bass_kernel_spmd(nc, in_maps, core_ids=list(range(NCORES)))
    return _gather_output([res.results[k]["y"] for k in range(NCORES)])


# revision 17
# speedup vs baseline: 1.3923x; 1.3923x over previous
"""Trainium2 Bass kernel for Conv2d: B=16, Cin=Cout=16, H=W=512, k=3, stride=1, pad=1.

Strategy:
  - Data-parallel over batch: 8 cores x 2 images each. Weights/bias replicated.
  - Per core the conv is a sequence of TensorEngine matmuls in an H-Toeplitz
    packing: contraction K = 16 ci x 8 input rows = 128, stationary
    M = 16 co x 6 output rows = 96, moving N = 512 w-pixels. Each chunk of 6
    output rows takes 3 matmuls (one per kw tap, column-shifted rhs)
    accumulating into one PSUM bank; kh lives inside the Toeplitz stationary.
  - Mixed-precision matmul: stationary weights fp16, moving x fp8 e3m4
    (1 cycle/col on PE either way). e3m4 x keeps rel-err ~1.3e-2 < 2e-2
    while halving input HBM traffic; output written back as fp16 (halves
    output traffic), upcast to fp32 on host.
  - Host-side gathered DRAM layouts:
      xg[b, ci, hi, j, w'] = xpad[b, ci, 6j+hi, w']   (8/6 row duplication)
      yg[b, co, ho, j, w]  -> y[b, co, 6j+ho, w]      (scattered back on host)
    so chunk-major group DMAs read/write multi-chunk contiguous runs per
    partition (grp=16 chunks per DMA => 8-16 KB descriptors per SDMA engine).
  - Partition layouts are channel-major (ci*8+hi / co*6+ho) and every DMA's
    DRAM-side outer dim is the 16-entry channel dim -> the HWDGE spreads each
    transfer across all 16 SDMA engines.
  - Input DMAs ride the sync HWDGE ring, output DMAs the scalar HWDGE ring.
  - Matmuls issue kw-major inside a sub-round of 8 chunks (all chunks' kw=1,
    then kw=0, then kw=2) so the stationary switches 3x per 8 chunks; the 8
    open PSUM accumulation groups occupy all 8 banks (single-buffered).
  - PSUM->SBUF copy + bias add alternates between the vector engine
    (tensor_scalar_add) and the scalar engine (activation Identity+bias) so
    neither engine gates the PE.
"""

import numpy as np

B, CIN, COUT, H, W = 16, 16, 16, 512, 512
NCORES = 8
BPC = B // NCORES  # images per core
T_OUT, T_IN = 6, 8
KP, MP = T_IN * CIN, T_OUT * COUT  # 128, 96
NCHUNK = (H + T_OUT - 1) // T_OUT  # 86
WPAD = W + 2  # 514 padded cols

DEFAULT_CFG = dict(
    mm_dtype="fp16",      # stationary weights dtype
    x_dtype="fp8e3",      # moving x dtype (fp8 e3m4)
    out_dtype="fp16",     # y HBM dtype
    in_dma="sync",
    out_dma="gpsimd",
    grp=16,               # chunks per input DMA group
    sub=8,                # chunks per PSUM sub-round (8 banks)
    out_grp=8,            # chunks per output DMA
    bias_split=True,      # alternate PSUM->SBUF+bias between vector/scalar
)

_cached = {}


def _groups(grp):
    # NCHUNK = 86 = 6 + 5*16: put the remainder group FIRST so the first
    # input DMA is small and the PE starts sooner.
    rem = NCHUNK % grp
    out = []
    j = 0
    if rem:
        out.append((0, rem))
        j = rem
    while j < NCHUNK:
        out.append((j, grp))
        j += grp
    return out


def _dt(mybir, name):
    return {
        "fp32": mybir.dt.float32,
        "fp32r": mybir.dt.float32r,
        "fp16": mybir.dt.float16,
        "bf16": mybir.dt.bfloat16,
        "fp8e3": mybir.dt.float8e3,
        "fp8e4": mybir.dt.float8e4,
    }[name]


def _build_program(**overrides):
    cfg = dict(DEFAULT_CFG, **overrides)
    key = tuple(sorted(cfg.items()))
    if key in _cached:
        return _cached[key]
    import concourse.bacc as bacc
    import concourse.tile as tile
    import concourse.mybir as mybir

    nc = bacc.Bacc(
        "TRN2",
        target_bir_lowering=False,
        debug=False,
        enable_asserts=False,
        num_devices=NCORES,
    )
    f32 = mybir.dt.float32
    wdt = _dt(mybir, cfg["mm_dtype"])
    xdt = _dt(mybir, cfg["x_dtype"])
    ydt = _dt(mybir, cfg["out_dtype"])
    x = nc.dram_tensor(
        "x", [BPC, CIN, T_IN, NCHUNK, WPAD], xdt, kind="ExternalInput"
    ).ap()
    wt = nc.dram_tensor("wt", [KP, 3 * MP], wdt, kind="ExternalInput").ap()
    bias = nc.dram_tensor("bias", [MP, 1], f32, kind="ExternalInput").ap()
    y = nc.dram_tensor(
        "y", [BPC, COUT, T_OUT, NCHUNK, W], ydt, kind="ExternalOutput"
    ).ap()

    in_eng = getattr(nc, cfg["in_dma"])
    out_eng = getattr(nc, cfg["out_dma"])
    grp = cfg["grp"]
    sub = cfg["sub"]
    ogrp = cfg["out_grp"]

    with tile.TileContext(nc) as tc:
        with (
            tc.tile_pool(name="consts", bufs=1) as cpool,
            tc.tile_pool(name="xin", bufs=4) as xpool,
            tc.tile_pool(name="psum", bufs=1, space="PSUM") as ppool,
            tc.tile_pool(name="outs", bufs=8) as opool,
        ):
            # constants ride the scalar ring so the first X tile's DMA
            # dispatches immediately on the sync ring
            wt_sb = cpool.tile([KP, 3 * MP], wdt)
            out_eng.dma_start(wt_sb[:], wt[:])
            bias_sb = cpool.tile([MP, 1], f32)
            out_eng.dma_start(bias_sb[:], bias[:])

            for b in range(BPC):
                for j0, g in _groups(grp):
                    X = xpool.tile([KP, grp * WPAD], xdt, tag="X")
                    # partition (ci*8+hi) <- g chunks, contiguous per
                    # partition in the gathered DRAM layout
                    in_eng.dma_start(
                        X[:, 0 : g * WPAD],
                        x[b, :, :, j0 : j0 + g, :],
                    )
                    for s0 in range(0, g, sub):
                        sg = min(sub, g - s0)
                        pss = [
                            ppool.tile([MP, W], f32, tag=f"ps{k}", name=f"ps{k}")
                            for k in range(sg)
                        ]
                        for i, kw in enumerate((1, 0, 2)):
                            for k in range(sg):
                                gi = s0 + k
                                nc.tensor.matmul(
                                    pss[k][:, :],
                                    wt_sb[:, kw * MP : (kw + 1) * MP],
                                    X[:, gi * WPAD + kw : gi * WPAD + kw + W],
                                    start=(i == 0),
                                    stop=(i == 2),
                                )
                        out_sb = None
                        for k in range(sg):
                            if k % ogrp == 0:
                                out_sb = opool.tile([MP, ogrp * W], ydt, tag="out")
                                o0 = k
                            dst = out_sb[:, (k - o0) * W : (k - o0 + 1) * W]
                            if cfg["bias_split"] and k % 2 == 1:
                                nc.scalar.activation(
                                    dst,
                                    pss[k][:, :],
                                    mybir.ActivationFunctionType.Identity,
                                    bias=bias_sb[:, 0:1],
                                    scale=1.0,
                                )
                            else:
                                nc.vector.tensor_scalar_add(
                                    dst, pss[k][:, :], bias_sb[:, 0:1]
                                )
                            if k % ogrp == ogrp - 1 or k == sg - 1:
                                # partition (co*6+ho) -> yg[b, co, ho, ...]
                                og = k - o0 + 1
                                out_eng.dma_start(
                                    y[b, :, :, j0 + s0 + o0 : j0 + s0 + o0 + og, :],
                                    out_sb[:, 0 : og * W],
                                )
    nc.compile()
    _cached[key] = nc
    return nc


def _np_dt(name):
    import ml_dtypes

    return {
        "fp32": np.float32,
        "fp32r": np.float32,
        "fp16": np.float16,
        "bf16": ml_dtypes.bfloat16,
        "fp8e3": ml_dtypes.float8_e3m4,
        "fp8e4": ml_dtypes.float8_e4m3,
    }[name]


def _toeplitz_weights(weights: np.ndarray) -> np.ndarray:
    """[COUT, CIN, 3, 3] -> [KP, 3*MP] with K index ci*T_IN+hi and M index
    co*T_OUT+ho; lhsT_kw[ci*8+hi, co*6+ho] = W[co, ci, hi-ho, kw] for
    0 <= hi-ho <= 2, else 0. kw blocks side by side."""
    wt = np.zeros((3, CIN, T_IN, COUT, T_OUT), dtype=np.float32)
    for kw in range(3):
        for ho in range(T_OUT):
            for kh in range(3):
                wt[kw, :, ho + kh, :, ho] = weights[:, :, kh, kw].T
    wt2 = wt.reshape(3, KP, MP)
    return np.ascontiguousarray(np.concatenate([wt2[0], wt2[1], wt2[2]], axis=1))


def _make_in_maps(x, weights, biases, mm_dtype=None):
    cfg = DEFAULT_CFG
    wnp = _np_dt(cfg["mm_dtype"])
    xnp = _np_dt(cfg["x_dtype"])
    wt_packed = _toeplitz_weights(weights).astype(wnp)
    bias_vec = np.ascontiguousarray(np.repeat(biases, T_OUT).reshape(MP, 1))
    # zero-pad to [HP, WPAD] then gather rows: xg[b,ci,hi,j,w] = xp[b,ci,6j+hi,w]
    hp = T_OUT * NCHUNK + 2  # 518
    xp = np.zeros((B, CIN, hp, WPAD), dtype=xnp)
    xp[:, :, 1 : 1 + H, 1 : 1 + W] = x.astype(xnp)
    rows = np.arange(T_IN)[:, None] + T_OUT * np.arange(NCHUNK)[None, :]  # [8, 86]
    xg = xp[:, :, rows, :]  # [B, CIN, 8, 86, WPAD]
    return [
        {
            "x": np.ascontiguousarray(xg[k * BPC : (k + 1) * BPC]),
            "wt": wt_packed,
            "bias": bias_vec,
        }
        for k in range(NCORES)
    ]


def _gather_output(res_list):
    yg = np.concatenate(res_list, axis=0)  # [B, COUT, 6, NCHUNK, W]
    yfull = yg.transpose(0, 1, 3, 2, 4).reshape(B, COUT, NCHUNK * T_OUT, W)
    return yfull[:, :, :H, :].astype(np.float32)


def kernel(x, weights, biases):
    from concourse import bass_utils

    x = np.ascontiguousarray(np.asarray(x, dtype=np.float32))
    weights = np.asarray(weights, dtype=np.float32)
    biases = np.asarray(biases, dtype=np.float32)

    nc = _build_program()
    in_maps = _make_in_maps(x, weights, biases)
    res = bass_utils.run_bass_kernel_spmd(nc, in_maps, core_ids=list(range(NCORES)))
    return _gather_output([res.results[k]["y"] for k in range(NCORES)])
